# revision 22
# baseline (speedup 1.0000x reference)
"""Trainium2 Bass kernel for nn_Alignment loss (CORAL-style alignment loss).

Strategy (hardcoded for B=64, hat_L=8, N=16, d=32, 8 cores):
  - Shard over hat_L: core i handles layer t=i (SPMD, per-core input shards).
  - All covariance Frobenius terms use the Gram trick:
      ||Xc^T Xc - Yc^T Yc||_F^2 = ||Xc Xc^T||^2 - 2||Xc Yc^T||^2 + ||Yc Yc^T||^2
    so the device only materializes 64x64 batch Grams, never feature covs.
  - The batch Gram is computed on RAW (uncentered) data in exact fp32; the
    rank-1 centering correction is applied on host in float64 from the raw
    inputs.  L_exo Grams are sums of per-t Grams (feature blocks).
  - The L_sfa tail (centering + transpose + per-node covariances) runs in
    bf16: its final-loss contribution is ~1%, so bf16 error is ~1e-5 on the
    output.  The 16x16 covariance inner products are done on host in
    float64 from the shipped bf16 C matrices.
  - E variance statistics use PE ones-matmuls on batch-major data; the
    ones column is embedded in the input/scratch images so every PE matmul
    carries at most one semaphore wait (hardware limit).
  - Inputs are host-packed into exact SBUF images (contiguous per-partition
    runs -> minimal DMA descriptor cost), one DMA per queue.
  - Device outputs per core: raw 2x2 block Gram [128,128] f32, bf16
    per-node covariances [32, 2*16*32], and E-sum/E-sumsq [128,8] f32.
"""

import numpy as np

import concourse.bass as bass
import concourse.tile as tile
from concourse import mybir
from concourse.bass_utils import run_bass_kernel_spmd

B = 64
T = 8
N = 16
D = 32
FW = N * D          # 512 flattened per-layer features
KCH = FW // 128     # 4 feature chunks of 128
ECH = (N * N) // 128  # 2 chunks for E features (256)
F32 = mybir.dt.float32
BF16 = mybir.dt.bfloat16

_BUILT = None


def _build():
    nc = bass.Bass()
    zz = nc.dram_tensor("zz", [128, KCH * 2 * B + 64], F32,
                        kind="ExternalInput")
    ee = nc.dram_tensor("ee", [B, 2 * N * N + 1], F32, kind="ExternalInput")
    out_o0 = nc.dram_tensor("out_o0", [128, 392], F32, kind="ExternalOutput")
    out_c1 = nc.dram_tensor("out_c1", [32, N * D], BF16,
                            kind="ExternalOutput")

    with tile.TileContext(nc) as tc:
        with tc.tile_pool(name="sb", bufs=1) as sb, \
             tc.tile_pool(name="ps1", bufs=1, space="PSUM") as ps1:
            # ---- loads: one packed image per DMA queue -------------------
            Zin = sb.tile([128, KCH * 2 * B + 64], F32)
            Zb = Zin[:, 0:KCH * 2 * B].rearrange("p (k s b) -> p k s b",
                                                 s=2, b=B)
            Ebm = sb.tile([B, 2 * N * N + 1], F32)  # batch-major E + ones
            nc.sync.dma_start(out=Zin[:, :], in_=zz[:])
            nc.scalar.dma_start(out=Ebm[:, :], in_=ee[:])

            # identity arrives packed (bf16 bits) in the Z image; fence it
            # through the DVE so transposes wait on a single (DVE) semaphore
            identity = sb.tile([128, 128], BF16)
            nc.vector.tensor_copy(
                out=identity[:, :],
                in_=Zin[:, KCH * 2 * B:].bitcast(BF16))
            # warm the ACT table for Copy while DMAs are in flight
            warm = sb.tile([1, 1], F32)
            nc.vector.memset(warm[:, :], 0.0)
            nc.scalar.copy(out=warm[:, :], in_=warm[:, :])

            # ---- center Z over batch -> bf16, per chunk-pair on DVE ------
            zsums = sb.tile([128, 2, 2, 2], F32)  # [p, pair, s, k2]
            Zc = sb.tile([128, KCH, 2, B], BF16)
            for pair in range(2):
                ks = slice(2 * pair, 2 * pair + 2)
                for s in range(2):
                    nc.vector.reduce_sum(out=zsums[:, pair, s, :],
                                         in_=Zb[:, ks, s, :],
                                         axis=mybir.AxisListType.X)
                    sums_b = zsums[:, pair, s, :].broadcast_to([128, 2, B])
                    nc.vector.scalar_tensor_tensor(
                        out=Zc[:, ks, s, :], in0=sums_b, scalar=-1.0 / B,
                        in1=Zb[:, ks, s, :], op0=mybir.AluOpType.mult,
                        op1=mybir.AluOpType.add)

            # ---- raw 2x2 block batch Gram [128,128] (fp32, exact) --------
            gpsum = ps1.tile([128, 128], F32)
            for k in range(KCH):
                blk = Zb[:, k, :, :].rearrange("p s b -> p (s b)")
                nc.tensor.matmul(gpsum[:, :], blk, blk,
                                 start=(k == 0), stop=(k == KCH - 1))

            # ---- transpose centered Z (bf16) to batch-major --------------
            # Zbm rows: 0-63 = Zsc [64, 512], 64-127 = Ztc [64, 512]
            Zbm = sb.tile([128, KCH, 128], BF16)
            for half in range(2):
                tp = ps1.tile([128, 2, 128], BF16, tag=f"tp{half}")
                for i in range(2):
                    k = half * 2 + i
                    blk = Zc[:, k, :, :].rearrange("p s b -> p (s b)")
                    nc.tensor.transpose(tp[:, i, :], blk, identity[:, :])
                dst = Zbm[:, 2 * half:2 * half + 2, :]
                if half == 0:
                    nc.vector.tensor_copy(out=dst, in_=tp[:, :, :])
                else:
                    nc.scalar.copy(out=dst, in_=tp[:, :, :])

            # ---- per-node covariances C[n] = Zc_n^T Zc_n [32,32] ---------
            cst0 = ps1.tile([32, N, D], F32)
            cst1 = ps1.tile([32, N, D], F32)
            O0 = sb.tile([128, 392], F32)
            nc.vector.memset(O0[:, 136:392], 0.0)
            STcat0 = O0[0:32, 136:392].bitcast(BF16).rearrange(
                "p (n b) -> p n b", n=N)
            STcat1 = sb.tile([32, N, D], BF16)
            for src in range(2):
                lo, hi = (0, B) if src == 0 else (B, 128)
                cst = cst0 if src == 0 else cst1
                for n in range(N):
                    k, c0 = divmod(n * D, 128)
                    lhs = Zbm[lo:hi, k, c0:c0 + D]
                    nc.tensor.matmul(cst[:, n, :], lhs, lhs,
                                     start=True, stop=True)
                # per-source copy (distinct PSUM tiles -> concurrent)
                if src == 0:
                    nc.vector.tensor_copy(out=STcat0[:, :, :],
                                          in_=cst0[:, :, :])
                else:
                    nc.scalar.copy(out=STcat1[:, :, :], in_=cst1[:, :, :])
            nc.scalar.dma_start(
                out=out_c1[:, :],
                in_=STcat1[:, :, :].rearrange("p n b -> p (n b)"))

            nc.vector.tensor_copy(out=O0[:, 0:128], in_=gpsum[:, :])

            # ---- E sums / sumsq via PE ones-matmuls (off the DVE) --------
            # Esq carries its own ones column so the sumsq matmuls wait on
            # the Pool sem only; the sums matmuls wait on the ee DMA only.
            Esq = sb.tile([B, 2 * N * N + 1], F32)
            nc.vector.tensor_mul(Esq[:, 0:2 * N * N],
                                 Ebm[:, 0:2 * N * N], Ebm[:, 0:2 * N * N])
            nc.vector.memset(Esq[:, 2 * N * N:], 1.0)
            epsum = ps1.tile([128, 2, ECH * 2], F32)
            ev = Ebm[:, 0:2 * N * N].rearrange("p (s f) -> p s f", s=2)
            qv = Esq[:, 0:2 * N * N].rearrange("p (s f) -> p s f", s=2)
            for s in range(2):
                for c in range(ECH):
                    nc.tensor.matmul(
                        epsum[:, 0, 2 * c + s:2 * c + s + 1],
                        ev[:, s, 128 * c:128 * (c + 1)],
                        Ebm[:, 2 * N * N:], start=True, stop=True)
                    nc.tensor.matmul(
                        epsum[:, 1, 2 * c + s:2 * c + s + 1],
                        qv[:, s, 128 * c:128 * (c + 1)],
                        Esq[:, 2 * N * N:], start=True, stop=True)
            nc.vector.tensor_copy(
                out=O0[:, 128:136].rearrange("p (u v) -> p u v", u=2),
                in_=epsum[:, :, :])
            nc.sync.dma_start(out=out_o0[:, :], in_=O0[:, :])

    return nc


def _get_nc():
    global _BUILT
    if _BUILT is None:
        _BUILT = _build()
    return _BUILT


def _prep_in_maps(Z_s, E_s, Z_t, E_t):
    in_maps = []
    for t in range(T):
        # Zb image: [128 p, k, s, b] = Z_src[b, 128k+p], split by chunk pair
        zzi = np.empty((128, KCH, 2, B), np.float32)
        zzi[:, :, 0, :] = Z_s[:, t].reshape(B, KCH, 128).transpose(2, 1, 0)
        zzi[:, :, 1, :] = Z_t[:, t].reshape(B, KCH, 128).transpose(2, 1, 0)
        # E image: batch-major [B, 2*256], plus a trailing ones column
        eei = np.empty((B, 2 * N * N + 1), np.float32)
        eei[:, 0:N * N] = E_s[:, t].reshape(B, N * N)
        eei[:, N * N:2 * N * N] = E_t[:, t].reshape(B, N * N)
        eei[:, 2 * N * N] = 1.0
        import ml_dtypes
        idbits = np.zeros((128, 128), ml_dtypes.bfloat16)
        np.fill_diagonal(idbits, 1.0)
        zfull = np.concatenate(
            [zzi.reshape(128, KCH * 2 * B),
             idbits.view(np.uint16).view(np.float32)], axis=1)
        in_maps.append({
            "zz": np.ascontiguousarray(zfull),
            "ee": np.ascontiguousarray(eei),
        })
    return in_maps


def _combine(results, Z_s, Z_t):
    """Host-side (float64) combine of per-core partial reductions."""
    LAM = 0.1
    EPS = 1e-8
    Bm1 = B - 1

    Gss_sum = np.zeros((B, B), np.float64)
    Gst_sum = np.zeros((B, B), np.float64)
    Gtt_sum = np.zeros((B, B), np.float64)
    W = np.zeros(T, np.float64)
    L_sca = np.zeros(T, np.float64)
    L_sfa = np.zeros(T, np.float64)

    for t in range(T):
        r = results[t]
        o0 = np.ascontiguousarray(r["out_o0"].reshape(128, 392))
        g = o0[:, 0:128].astype(np.float64)
        # exact rank-1 centering corrections from the raw inputs
        Xs = Z_s[:, t].reshape(B, FW).astype(np.float64)
        Xt = Z_t[:, t].reshape(B, FW).astype(np.float64)
        mus, mut = Xs.mean(0), Xt.mean(0)
        Gss = g[:B, :B] - np.add.outer(Xs @ mus, Xs @ mus) + (mus @ mus)
        Gst = g[:B, B:] - np.add.outer(Xs @ mut, Xt @ mus) + (mus @ mut)
        Gtt = g[B:, B:] - np.add.outer(Xt @ mut, Xt @ mut) + (mut @ mut)
        Gss_sum += Gss
        Gst_sum += Gst
        Gtt_sum += Gtt
        num = (Gss * Gss).sum() - 2.0 * (Gst * Gst).sum() + (Gtt * Gtt).sum()
        W[t] = num / (Bm1 * Bm1 * 4.0 * FW * FW)

        # C matrices: [a, (n, b)] = C_src[n, a, b] (bf16)
        import ml_dtypes
        c0 = o0[0:32, 136:392].view(ml_dtypes.bfloat16).astype(
            np.float64).reshape(32, N, D)
        c1 = np.asarray(r["out_c1"]).astype(np.float64).reshape(32, N, D)
        Cs = c0.transpose(1, 0, 2) / Bm1   # [n, a, b]
        Ct = c1.transpose(1, 0, 2) / Bm1
        ss = np.einsum("nab,nab->n", Cs, Cs)
        tt = np.einsum("nab,nab->n", Ct, Ct)
        st = np.einsum("nab,jab->nj", Cs, Ct)
        Dm = (ss[:, None] + tt[None, :] - 2.0 * st) / (4.0 * D * D)
        pos = np.diag(Dm)
        neg = Dm.sum(axis=1) - pos
        L_sfa[t] = np.mean(np.log(np.exp(pos) + neg + EPS) - pos)

        e = o0[:, 128:136].astype(np.float64).reshape(128, 2, ECH * 2)
        sums = e[:, 0, :].reshape(128, ECH, 2)
        sumsq = e[:, 1, :].reshape(128, ECH, 2)
        var = (sumsq - sums * sums / B) / Bm1
        dv = var[:, :, 0] - var[:, :, 1]
        L_sca[t] = np.mean(dv * dv) / 4.0

    fexo = T * FW
    num = ((Gss_sum * Gss_sum).sum() - 2.0 * (Gst_sum * Gst_sum).sum()
           + (Gtt_sum * Gtt_sum).sum())
    L_exo = num / (Bm1 * Bm1 * 4.0 * fexo * fexo)
    L_iendo = float((W * (LAM * L_sca + LAM * L_sfa)).sum())
    return np.float32(L_exo + L_iendo / T)


def _run(Z_s, E_s, Z_t, E_t, trace=False, **kw):
    nc = _get_nc()
    in_maps = _prep_in_maps(Z_s, E_s, Z_t, E_t)
    res = run_bass_kernel_spmd(nc, in_maps, core_ids=list(range(T)),
                               trace=trace, **kw)
    return _combine(res.results, Z_s, Z_t), res


def kernel(Z_s, E_s, Z_t, E_t):
    out, _ = _run(Z_s, E_s, Z_t, E_t)
    return out
